# revision 16
# baseline (speedup 1.0000x reference)
"""AdditiveAttention (d2l-style) distributed Bass kernel for 8 TRN2 NeuronCores.

Full inputs in, full output out. Sharding: batch b = core//2, query-row half
core%2 (128 rows each). Each core runs an identical fused kernel:

  qT[h,lq] = (q @ Wq).T, kT[h,lk] = (k @ Wk).T          (PE, d-tiled)
  pre[h, (lk,lq)] = qT[h,lq] + kT[h,lk]                 (DVE adds, k col as
                                                         per-partition scalar)
  feat = tanh(pre)                                      (ACT, big chunks)
  scores[lq, lk] = wv . feat[:, lk, :]                  (PE: feat block as
                                                         stationary, wv moving,
                                                         N=1 -> one scores
                                                         column per matmul)
  p = exp(scores)          (no max subtraction needed: |scores| <= sum|wv| ~ 9,
                            exp can't overflow; softmax shift cancels exactly)
  out = (p @ (v ⊙ m)) / (p @ m)                         (PE transposes of p +
                                                         accumulated matmuls)

The valid-length mask enters as a 0/1 column per key position; zeroing masked
value rows and masked denominator terms is exactly equivalent to the
reference's -1e6 additive masking. The feat pipeline runs in bf16 (DVE 4x
mode, PE fast-weight-load); accumulations stay fp32 in PSUM.
"""

import sys

sys.path.insert(0, "/opt/trn_rl_repo")

from contextlib import ExitStack

import numpy as np

import concourse.bass as bass
import concourse.mybir as mybir
from concourse import bass_utils, tile

B, LQ, LK, DQ, DK, DV, H = 4, 256, 512, 256, 256, 256, 128
NCORES = 8
ROWS = LQ // 2  # lq rows per core = 128
W = 64  # lk columns per tanh chunk
F32 = mybir.dt.float32
BF16 = mybir.dt.bfloat16
AF = mybir.ActivationFunctionType

# packed input blob column layouts (see _body)
BLOB_A_COLS = 2 * ROWS + 2 * LK + 4 * H  # qT(2), kT(2), Wq(2), Wk(2) = 1792
BLOB_B_COLS = 1 + 4 * DV + 4 + 128  # wv, v(4), m(4), ident = 1157


def _body(ctx: ExitStack, tc: "tile.TileContext", aps: dict):
    nc = tc.nc
    const = ctx.enter_context(tc.tile_pool(name="const", bufs=1))
    work = ctx.enter_context(tc.tile_pool(name="work", bufs=1))
    proj_ps = ctx.enter_context(tc.tile_pool(name="proj_ps", bufs=2, space="PSUM"))
    t_ps = ctx.enter_context(tc.tile_pool(name="t_ps", bufs=2, space="PSUM"))
    acc_ps = ctx.enter_context(tc.tile_pool(name="acc_ps", bufs=1, space="PSUM"))

    # Two packed input blobs, one DMA each (single HWDGE queue/semaphore per
    # blob so no consumer ever needs waits on two DMA queues — walrus's
    # LDWEIGHTS struct only carries one sync-wait). Blob A feeds the PE
    # projections; blob B feeds everything else (via DVE casts).
    blob_a = const.tile([128, BLOB_A_COLS], F32, tag="blob_a")
    nc.sync.dma_start(blob_a[:], aps["blob_a"][:, :])
    blob_b = const.tile([128, BLOB_B_COLS], F32, tag="blob_b")
    nc.sync.dma_start(blob_b[:], aps["blob_b"][:, :])

    def slab(blob, off, w):
        return blob[:, off : off + w]

    qT_in = [slab(blob_a, 0, ROWS), slab(blob_a, 128, ROWS)]
    kT_in = [slab(blob_a, 256, LK), slab(blob_a, 768, LK)]
    Wq_sb = [slab(blob_a, 1280, H), slab(blob_a, 1408, H)]
    Wk_sb = [slab(blob_a, 1536, H), slab(blob_a, 1664, H)]
    wv_sb = slab(blob_b, 0, 1)
    v_sb = [slab(blob_b, 1 + 256 * t, DV) for t in range(4)]
    m_sb = [slab(blob_b, 1025 + t, 1) for t in range(4)]
    id_sb = slab(blob_b, 1029, 128)

    # bf16 casts of small constants
    wv_bf = const.tile([H, 1], BF16, tag="wv_bf")
    nc.vector.tensor_copy(wv_bf[:], wv_sb)
    id_bf = const.tile([128, 128], BF16, tag="id_bf")
    nc.vector.tensor_copy(id_bf[:], id_sb)
    mb, vb = [], []
    for t in range(4):
        # mask + cast in one op: vb[lk, :] = v[lk, :] * m[lk]
        x = const.tile([128, DV], BF16, tag=f"vb{t}")
        nc.vector.tensor_scalar_mul(x[:], v_sb[t], m_sb[t])
        vb.append(x)
        x = const.tile([128, 1], BF16, tag=f"mb{t}")
        nc.vector.tensor_copy(x[:], m_sb[t])
        mb.append(x)

    # Projections: kT[h, lk] = sum_d Wk[d, h] * kT_in[d, lk] (2 d-tiles)
    kT_p = proj_ps.tile([H, LK], F32, tag="proj")
    nc.tensor.matmul(kT_p[:], lhsT=Wk_sb[0], rhs=kT_in[0], start=True, stop=False)
    nc.tensor.matmul(kT_p[:], lhsT=Wk_sb[1], rhs=kT_in[1], start=False, stop=True)
    # stays f32: read back as the per-partition scalar operand of the adds
    kTf = const.tile([H, LK], F32, tag="kTf")
    nc.vector.tensor_copy(kTf[:], kT_p[:])

    qT_p = proj_ps.tile([H, ROWS], F32, tag="proj")
    nc.tensor.matmul(qT_p[:], lhsT=Wq_sb[0], rhs=qT_in[0], start=True, stop=False)
    nc.tensor.matmul(qT_p[:], lhsT=Wq_sb[1], rhs=qT_in[1], start=False, stop=True)
    qTb = const.tile([H, ROWS], BF16, tag="qTb")
    nc.vector.tensor_copy(qTb[:], qT_p[:])

    # Main loop over lk chunks: pre[h, (lk, lq)] = qT[h, :] + kT[h, lk];
    # feat = tanh(pre) IN PLACE; scores[:, lk] = feat_lk.T @ wv (one column
    # per matmul). Every chunk gets its own never-reused buffer so no
    # instruction ever carries WAR/WAW waits (walrus only allows one
    # sync-wait per instruction).
    scores_ps = acc_ps.tile([ROWS, LK], F32, tag="scores")
    for c in range(LK // W):
        feat = work.tile([H, W * ROWS], BF16, tag=f"ch{c}")
        for j in range(W):
            lk = c * W + j
            nc.vector.tensor_scalar_add(
                feat[:, ROWS * j : ROWS * (j + 1)], qTb[:], kTf[:, lk : lk + 1]
            )
        nc.scalar.activation(feat[:], feat[:], AF.Tanh)
        for j in range(W):
            lk = c * W + j
            nc.tensor.matmul(
                scores_ps[0:ROWS, lk : lk + 1],
                lhsT=feat[:, ROWS * j : ROWS * (j + 1)],
                rhs=wv_bf[:],
                start=True,
                stop=True,
            )

    # p = exp(scores); tanh-bounded scores can't overflow exp, and softmax's
    # max-shift cancels in p/sum, so no row-max pass is needed.
    p_sb = const.tile([ROWS, LK], BF16, tag="p")
    nc.scalar.activation(p_sb[:], scores_ps[:], AF.Exp)

    pT_sb = []
    for t in range(4):
        pT_p = t_ps.tile([128, 128], BF16, tag="pT_ps")
        nc.tensor.transpose(pT_p[:], p_sb[:, 128 * t : 128 * (t + 1)], id_bf[:])
        s = work.tile([128, 128], BF16, tag=f"pT_sb{t}")
        nc.vector.tensor_copy(s[:], pT_p[:])
        pT_sb.append(s)

    out_ps = acc_ps.tile([ROWS, DV], F32, tag="out_ps")
    for t in range(4):
        nc.tensor.matmul(out_ps[:], lhsT=pT_sb[t][:], rhs=vb[t][:], start=(t == 0), stop=(t == 3))
    sum_ps = acc_ps.tile([ROWS, 1], F32, tag="sum_ps")
    for t in range(4):
        nc.tensor.matmul(sum_ps[:], lhsT=pT_sb[t][:], rhs=mb[t][:], start=(t == 0), stop=(t == 3))

    rs = const.tile([ROWS, 1], F32, tag="rs")
    nc.vector.reciprocal(rs[:], sum_ps[:])
    out_sb = const.tile([ROWS, DV], F32, tag="out_sb")
    nc.vector.tensor_scalar_mul(out_sb[:], out_ps[:], rs[:, 0:1])
    nc.sync.dma_start(aps["out"][:, :], out_sb[:])


def build_graph() -> bass.Bass:
    nc = bass.Bass("TRN2", target_bir_lowering=False, debug=False)

    def inp(name, shape):
        return nc.dram_tensor(name, shape, F32, kind="ExternalInput").ap()

    aps = {
        "blob_a": inp("blob_a", [128, BLOB_A_COLS]),
        "blob_b": inp("blob_b", [128, BLOB_B_COLS]),
        "out": nc.dram_tensor("out", [ROWS, DV], F32, kind="ExternalOutput").ap(),
    }
    with tile.TileContext(nc) as tc:
        with ExitStack() as ctx:
            _body(ctx, tc, aps)
    _split_multi_waits(nc)
    return nc


def _split_multi_waits(nc):
    """This walrus build accepts only ONE sync-wait per instruction (every
    TPB struct's setupSyncWait rejects more). Tile emits instructions with
    several waits. Legalize: keep one wait on the instruction and hoist the
    rest onto freshly inserted same-engine NOPs placed immediately before it
    in the basic block — identical blocking semantics, no reordering."""
    n = 0
    for bb in nc.m.functions[0].blocks:
        insts = bb.instructions
        out = []
        for inst in insts:
            si = inst.sync_info
            if si is not None and si.on_wait and len(si.on_wait) > 1:
                waits = list(si.on_wait)
                for w in waits[:-1]:
                    nop = mybir.InstNoOp(
                        name=f"{inst.name}-wsplit{n}",
                        text_hint="waitsplit",
                        bass_nofuse=True,
                        engine=inst.engine,
                        sync_info=mybir.SyncInfo(on_wait=[w], on_update=[]),
                    )
                    nc.register_instruction(nop)
                    out.append(nop)
                    n += 1
                inst.sync_info = mybir.SyncInfo(
                    on_wait=[waits[-1]], on_update=si.on_update
                )
            out.append(inst)
        if n:
            bb.instructions = out


def make_in_maps(queries, keys, values, Wq, Wk, wv, valid_lens):
    f = np.float32
    queries = np.asarray(queries, f)
    keys = np.asarray(keys, f)
    values = np.asarray(values, f)
    Wqf = np.asarray(Wq, f)
    Wkf = np.asarray(Wk, f)
    wvf = np.asarray(wv, f).reshape(H)
    ident = np.eye(128, dtype=f)
    in_maps = []
    for c in range(NCORES):
        b, r0 = c // 2, (c % 2) * ROWS
        m = (np.arange(LK) < int(valid_lens[b])).astype(f)

        a = np.empty((128, BLOB_A_COLS), f)
        qT = queries[b, r0 : r0 + ROWS, :].T  # [DQ, ROWS]
        kT = keys[b].T  # [DK, LK]
        a[:, 0:128] = qT[0:128]
        a[:, 128:256] = qT[128:256]
        a[:, 256:768] = kT[0:128]
        a[:, 768:1280] = kT[128:256]
        a[:, 1280:1408] = Wqf[0:128]
        a[:, 1408:1536] = Wqf[128:256]
        a[:, 1536:1664] = Wkf[0:128]
        a[:, 1664:1792] = Wkf[128:256]

        bb = np.empty((128, BLOB_B_COLS), f)
        bb[:, 0] = wvf
        for t in range(4):
            bb[:, 1 + 256 * t : 1 + 256 * (t + 1)] = values[b, 128 * t : 128 * (t + 1), :]
            bb[:, 1025 + t] = m[128 * t : 128 * (t + 1)]
        bb[:, 1029:1157] = ident

        in_maps.append({"blob_a": a, "blob_b": bb})
    return in_maps


_CACHE: dict = {}


def kernel(queries, keys, values, Wq, Wk, wv, valid_lens, _trace=False, _trace_kwargs=None):
    if "nc" not in _CACHE:
        _CACHE["nc"] = build_graph()
    nc = _CACHE["nc"]
    in_maps = make_in_maps(queries, keys, values, Wq, Wk, wv, valid_lens)
    res = bass_utils.run_bass_kernel_spmd(
        nc,
        in_maps,
        core_ids=list(range(NCORES)),
        trace=_trace,
        **(_trace_kwargs or {}),
    )
    out = np.empty((B, LQ, DV), dtype=np.float32)
    for c in range(NCORES):
        b, r0 = c // 2, (c % 2) * ROWS
        out[b, r0 : r0 + ROWS, :] = res.results[c]["out"]
    if _trace:
        return out, res
    return out


# revision 21
# speedup vs baseline: 1.3560x; 1.3560x over previous
"""AdditiveAttention (d2l-style) distributed Bass kernel for 8 TRN2 NeuronCores.

Full inputs in, full output out. Sharding: batch b = core//2, query-row half
core%2 (128 rows each). Each core runs an identical fused kernel:

  qT[h,lq] = (q @ Wq).T, kT[h,lk] = (k @ Wk).T          (PE, d-tiled)
  feat[h, (lq,lk)] = tanh(kT[h,:] + qT[h,lq])           (DVE adds FD=512 with
                                                         q col as per-partition
                                                         scalar; ACT tanh in
                                                         place on big chunks)
  scoresT[lk, lq] = wv . feat[:, lq, lk-block]          (PE: feat block as
                                                         stationary, wv moving,
                                                         N=1 -> one scoresT
                                                         column per matmul)
  pT = exp(scoresT)        (no max subtraction needed: |scores| <= sum|wv| ~ 9,
                            exp can't overflow; softmax shift cancels exactly)
  out = (pT.T @ (v ⊙ m)) / (pT.T @ m)                   (accumulated matmuls,
                                                         pT already transposed)

The valid-length mask enters as a 0/1 column per key position; zeroing masked
value rows and masked denominator terms is exactly equivalent to the
reference's -1e6 additive masking. The feat pipeline runs in bf16 (DVE 4x
mode, PE fast-weight-load); accumulations stay fp32 in PSUM.
"""

import sys

sys.path.insert(0, "/opt/trn_rl_repo")

from contextlib import ExitStack

import numpy as np

import concourse.bass as bass
import concourse.mybir as mybir
from concourse import bass_utils, tile

B, LQ, LK, DQ, DK, DV, H = 4, 256, 512, 256, 256, 256, 128
NCORES = 8
ROWS = LQ // 2  # lq rows per core = 128
G = 16  # lq columns per tanh chunk
F32 = mybir.dt.float32
BF16 = mybir.dt.bfloat16
AF = mybir.ActivationFunctionType

# packed input blob column layouts (see _body)
BLOB_A_COLS = 2 * ROWS + 2 * LK + 4 * H  # qT(2), kT(2), Wq(2), Wk(2) = 1792
BLOB_B_COLS = 1 + 4 * DV + 4  # wv, v(4), m(4) = 1029


def _body(ctx: ExitStack, tc: "tile.TileContext", aps: dict):
    nc = tc.nc
    const = ctx.enter_context(tc.tile_pool(name="const", bufs=1))
    work = ctx.enter_context(tc.tile_pool(name="work", bufs=1))
    proj_ps = ctx.enter_context(tc.tile_pool(name="proj_ps", bufs=2, space="PSUM"))
    acc_ps = ctx.enter_context(tc.tile_pool(name="acc_ps", bufs=1, space="PSUM"))

    # Two packed input blobs, one DMA each (single HWDGE queue/semaphore per
    # blob so no consumer ever needs waits on two DMA queues — walrus's
    # LDWEIGHTS struct only carries one sync-wait). Blob A feeds the PE
    # projections; blob B feeds everything else (via DVE casts).
    blob_a = const.tile([128, BLOB_A_COLS], F32, tag="blob_a")
    nc.sync.dma_start(blob_a[:], aps["blob_a"][:, :])
    blob_b = const.tile([128, BLOB_B_COLS], F32, tag="blob_b")
    nc.sync.dma_start(blob_b[:], aps["blob_b"][:, :])

    def slab(blob, off, w):
        return blob[:, off : off + w]

    qT_in = [slab(blob_a, 0, ROWS), slab(blob_a, 128, ROWS)]
    kT_in = [slab(blob_a, 256, LK), slab(blob_a, 768, LK)]
    Wq_sb = [slab(blob_a, 1280, H), slab(blob_a, 1408, H)]
    Wk_sb = [slab(blob_a, 1536, H), slab(blob_a, 1664, H)]
    wv_sb = slab(blob_b, 0, 1)
    v_sb = [slab(blob_b, 1 + 256 * t, DV) for t in range(4)]
    m_sb = [slab(blob_b, 1025 + t, 1) for t in range(4)]

    # bf16 casts of small constants
    wv_bf = const.tile([H, 1], BF16, tag="wv_bf")
    nc.vector.tensor_copy(wv_bf[:], wv_sb)
    mb, vb = [], []
    for t in range(4):
        # mask + cast in one op: vb[lk, :] = v[lk, :] * m[lk]
        x = const.tile([128, DV], BF16, tag=f"vb{t}")
        nc.vector.tensor_scalar_mul(x[:], v_sb[t], m_sb[t])
        vb.append(x)
        x = const.tile([128, 1], BF16, tag=f"mb{t}")
        nc.vector.tensor_copy(x[:], m_sb[t])
        mb.append(x)

    # Projections: kT[h, lk] = sum_d Wk[d, h] * kT_in[d, lk] (2 d-tiles)
    kT_p = proj_ps.tile([H, LK], F32, tag="proj")
    nc.tensor.matmul(kT_p[:], lhsT=Wk_sb[0], rhs=kT_in[0], start=True, stop=False)
    nc.tensor.matmul(kT_p[:], lhsT=Wk_sb[1], rhs=kT_in[1], start=False, stop=True)
    kTb = const.tile([H, LK], BF16, tag="kTb")
    nc.vector.tensor_copy(kTb[:], kT_p[:])

    qT_p = proj_ps.tile([H, ROWS], F32, tag="proj")
    nc.tensor.matmul(qT_p[:], lhsT=Wq_sb[0], rhs=qT_in[0], start=True, stop=False)
    nc.tensor.matmul(qT_p[:], lhsT=Wq_sb[1], rhs=qT_in[1], start=False, stop=True)
    # stays f32: read back as the per-partition scalar operand of the adds
    qTf = const.tile([H, ROWS], F32, tag="qTf")
    nc.vector.tensor_copy(qTf[:], qT_p[:])

    # Main loop, q-major: feat[h, (lq, lk)] = tanh(kT[h, :] + qT[h, lq]) with
    # the whole kT row-block as the streamed operand (FD=512 per DVE op) and
    # the q column as per-partition scalar; tanh IN PLACE on big chunks.
    # Then one column-matvec per (lq, lk-block): lhsT = feat slice [h, 128
    # contiguous lk], rhs = wv -> out [128(lk), 1] = column lq of scoresT
    # tile t. Every chunk buffer is never reused (no WAR/WAW waits; walrus
    # only allows one sync-wait per instruction).
    scoresT_ps = []
    for t in range(4):
        sc = acc_ps.tile([128, ROWS], F32, tag=f"scT{t}")
        scoresT_ps.append(sc)
    for c in range(ROWS // G):
        feat = work.tile([H, G * LK], BF16, tag=f"ch{c}")
        for g in range(G):
            lq = c * G + g
            nc.vector.tensor_scalar_add(
                feat[:, LK * g : LK * (g + 1)], kTb[:], qTf[:, lq : lq + 1]
            )
        nc.scalar.activation(feat[:], feat[:], AF.Tanh)
        for g in range(G):
            lq = c * G + g
            for t in range(4):
                nc.tensor.matmul(
                    scoresT_ps[t][0:128, lq : lq + 1],
                    lhsT=feat[:, LK * g + 128 * t : LK * g + 128 * (t + 1)],
                    rhs=wv_bf[:],
                    start=True,
                    stop=True,
                )

    # pT = exp(scoresT) directly in transposed layout (tanh-bounded scores
    # can't overflow exp, and softmax's max-shift cancels in p/sum, so no
    # row-max pass is needed).
    pT_sb = []
    for t in range(4):
        s = work.tile([128, ROWS], BF16, tag=f"pT_sb{t}")
        nc.scalar.activation(s[:], scoresT_ps[t][:], AF.Exp)
        pT_sb.append(s)

    out_ps = acc_ps.tile([ROWS, DV], F32, tag="out_ps")
    for t in range(4):
        nc.tensor.matmul(out_ps[:], lhsT=pT_sb[t][:], rhs=vb[t][:], start=(t == 0), stop=(t == 3))
    sum_ps = acc_ps.tile([ROWS, 1], F32, tag="sum_ps")
    for t in range(4):
        nc.tensor.matmul(sum_ps[:], lhsT=pT_sb[t][:], rhs=mb[t][:], start=(t == 0), stop=(t == 3))

    rs = const.tile([ROWS, 1], F32, tag="rs")
    nc.vector.reciprocal(rs[:], sum_ps[:])
    out_sb = const.tile([ROWS, DV], F32, tag="out_sb")
    nc.vector.tensor_scalar_mul(out_sb[:], out_ps[:], rs[:, 0:1])
    nc.sync.dma_start(aps["out"][:, :], out_sb[:])


def build_graph() -> bass.Bass:
    nc = bass.Bass("TRN2", target_bir_lowering=False, debug=False)

    def inp(name, shape):
        return nc.dram_tensor(name, shape, F32, kind="ExternalInput").ap()

    aps = {
        "blob_a": inp("blob_a", [128, BLOB_A_COLS]),
        "blob_b": inp("blob_b", [128, BLOB_B_COLS]),
        "out": nc.dram_tensor("out", [ROWS, DV], F32, kind="ExternalOutput").ap(),
    }
    with tile.TileContext(nc) as tc:
        with ExitStack() as ctx:
            _body(ctx, tc, aps)
    _split_multi_waits(nc)
    return nc


def _split_multi_waits(nc):
    """This walrus build accepts only ONE sync-wait per instruction (every
    TPB struct's setupSyncWait rejects more). Tile emits instructions with
    several waits. Legalize: keep one wait on the instruction and hoist the
    rest onto freshly inserted same-engine NOPs placed immediately before it
    in the basic block — identical blocking semantics, no reordering."""
    n = 0
    for bb in nc.m.functions[0].blocks:
        insts = bb.instructions
        out = []
        for inst in insts:
            si = inst.sync_info
            if si is not None and si.on_wait and len(si.on_wait) > 1:
                waits = list(si.on_wait)
                for w in waits[:-1]:
                    nop = mybir.InstNoOp(
                        name=f"{inst.name}-wsplit{n}",
                        text_hint="waitsplit",
                        bass_nofuse=True,
                        engine=inst.engine,
                        sync_info=mybir.SyncInfo(on_wait=[w], on_update=[]),
                    )
                    nc.register_instruction(nop)
                    out.append(nop)
                    n += 1
                inst.sync_info = mybir.SyncInfo(
                    on_wait=[waits[-1]], on_update=si.on_update
                )
            out.append(inst)
        if n:
            bb.instructions = out


def make_in_maps(queries, keys, values, Wq, Wk, wv, valid_lens):
    f = np.float32
    queries = np.asarray(queries, f)
    keys = np.asarray(keys, f)
    values = np.asarray(values, f)
    Wqf = np.asarray(Wq, f)
    Wkf = np.asarray(Wk, f)
    wvf = np.asarray(wv, f).reshape(H)
    in_maps = []
    for c in range(NCORES):
        b, r0 = c // 2, (c % 2) * ROWS
        m = (np.arange(LK) < int(valid_lens[b])).astype(f)

        a = np.empty((128, BLOB_A_COLS), f)
        qT = queries[b, r0 : r0 + ROWS, :].T  # [DQ, ROWS]
        kT = keys[b].T  # [DK, LK]
        a[:, 0:128] = qT[0:128]
        a[:, 128:256] = qT[128:256]
        a[:, 256:768] = kT[0:128]
        a[:, 768:1280] = kT[128:256]
        a[:, 1280:1408] = Wqf[0:128]
        a[:, 1408:1536] = Wqf[128:256]
        a[:, 1536:1664] = Wkf[0:128]
        a[:, 1664:1792] = Wkf[128:256]

        bb = np.empty((128, BLOB_B_COLS), f)
        bb[:, 0] = wvf
        for t in range(4):
            bb[:, 1 + 256 * t : 1 + 256 * (t + 1)] = values[b, 128 * t : 128 * (t + 1), :]
            bb[:, 1025 + t] = m[128 * t : 128 * (t + 1)]

        in_maps.append({"blob_a": a, "blob_b": bb})
    return in_maps


_CACHE: dict = {}


def kernel(queries, keys, values, Wq, Wk, wv, valid_lens, _trace=False, _trace_kwargs=None):
    if "nc" not in _CACHE:
        _CACHE["nc"] = build_graph()
    nc = _CACHE["nc"]
    in_maps = make_in_maps(queries, keys, values, Wq, Wk, wv, valid_lens)
    res = bass_utils.run_bass_kernel_spmd(
        nc,
        in_maps,
        core_ids=list(range(NCORES)),
        trace=_trace,
        **(_trace_kwargs or {}),
    )
    out = np.empty((B, LQ, DV), dtype=np.float32)
    for c in range(NCORES):
        b, r0 = c // 2, (c % 2) * ROWS
        out[b, r0 : r0 + ROWS, :] = res.results[c]["out"]
    if _trace:
        return out, res
    return out


# revision 26
# speedup vs baseline: 1.4935x; 1.1014x over previous
"""AdditiveAttention (d2l-style) distributed Bass kernel for 8 TRN2 NeuronCores.

Full inputs in, full output out. Sharding: batch b = core//2, query-row half
core%2 (128 rows each). Each core runs an identical fused kernel:

  qT[h,lq] = (q @ Wq).T, kT[h,lk] = (k @ Wk).T          (PE, d-tiled)
  feat[h, (lq,lk)] = tanh(kT[h,:] + qT[h,lq])           (DVE adds FD=512 with
                                                         q col as per-partition
                                                         scalar; ACT tanh in
                                                         place on big chunks)
  scoresT[lk, lq] = wv . feat[:, lq, lk-block]          (PE: feat block as
                                                         stationary, wv moving,
                                                         N=1 -> one scoresT
                                                         column per matmul)
  pT = exp(scoresT)        (no max subtraction needed: |scores| <= sum|wv| ~ 9,
                            exp can't overflow; softmax shift cancels exactly)
  out = (pT.T @ (v ⊙ m)) / (pT.T @ m)                   (accumulated matmuls,
                                                         pT already transposed)

The valid-length mask enters as a 0/1 column per key position; zeroing masked
value rows and masked denominator terms is exactly equivalent to the
reference's -1e6 additive masking. The feat pipeline runs in bf16 (DVE 4x
mode, PE fast-weight-load); accumulations stay fp32 in PSUM.
"""

import sys

sys.path.insert(0, "/opt/trn_rl_repo")

from contextlib import ExitStack

import numpy as np

import concourse.bass as bass
import concourse.mybir as mybir
from concourse import bass_utils, tile

B, LQ, LK, DQ, DK, DV, H = 4, 256, 512, 256, 256, 256, 128
NCORES = 8
ROWS = LQ // 2  # lq rows per core = 128
G = 16  # lq columns per tanh chunk
F32 = mybir.dt.float32
BF16 = mybir.dt.bfloat16
AF = mybir.ActivationFunctionType

# packed all-bf16 input blob: qT(2), kT(2), Wq(2), Wk(2), wv, v(4), m(4)
BLOB_COLS = 2 * ROWS + 2 * LK + 4 * H + 1 + 4 * DV + 4  # = 2821
# tanh chunk sizes (lq per chunk): small leading chunks let the first tanh
# start after only a few DVE adds; small trailing chunks shorten the tail
# before the exp/output phase.
CHUNKS = [4, 4, 8] + [16] * 6 + [8, 4, 4]
assert sum(CHUNKS) == ROWS


def _body(ctx: ExitStack, tc: "tile.TileContext", aps: dict):
    nc = tc.nc
    const = ctx.enter_context(tc.tile_pool(name="const", bufs=1))
    work = ctx.enter_context(tc.tile_pool(name="work", bufs=1))
    proj_ps = ctx.enter_context(tc.tile_pool(name="proj_ps", bufs=2, space="PSUM"))
    acc_ps = ctx.enter_context(tc.tile_pool(name="acc_ps", bufs=1, space="PSUM"))

    # One packed all-bf16 input blob, one DMA (single HWDGE queue/semaphore so
    # no consumer ever needs waits on two DMA queues — walrus only carries one
    # sync-wait per instruction). bf16 halves DMA bytes and makes the
    # projection matmuls run at bf16 speed (fp32 matmul is ~5.7x slower).
    blob = const.tile([128, BLOB_COLS], BF16, tag="blob")
    nc.sync.dma_start(blob[:], aps["blob"][:, :])

    def slab(off, w):
        return blob[:, off : off + w]

    qT_in = [slab(0, ROWS), slab(128, ROWS)]
    kT_in = [slab(256, LK), slab(768, LK)]
    Wq_sb = [slab(1280, H), slab(1408, H)]
    Wk_sb = [slab(1536, H), slab(1664, H)]
    wv_bf = slab(1792, 1)
    v_sb = [slab(1793 + 256 * t, DV) for t in range(4)]
    mb = [slab(2817 + t, 1) for t in range(4)]

    # masked values: vb[lk, :] = v[lk, :] * m[lk] (m broadcast along free dim)
    vb = []
    for t in range(4):
        x = const.tile([128, DV], BF16, tag=f"vb{t}")
        nc.vector.tensor_tensor(
            out=x[:], in0=v_sb[t], in1=mb[t].broadcast_to([128, DV]), op=mybir.AluOpType.mult
        )
        vb.append(x)

    # Projections: kT[h, lk] = sum_d Wk[d, h] * kT_in[d, lk] (2 d-tiles)
    kT_p = proj_ps.tile([H, LK], F32, tag="proj")
    nc.tensor.matmul(kT_p[:], lhsT=Wk_sb[0], rhs=kT_in[0], start=True, stop=False)
    nc.tensor.matmul(kT_p[:], lhsT=Wk_sb[1], rhs=kT_in[1], start=False, stop=True)
    kTb = const.tile([H, LK], BF16, tag="kTb")
    nc.vector.tensor_copy(kTb[:], kT_p[:])

    qT_p = proj_ps.tile([H, ROWS], F32, tag="proj")
    nc.tensor.matmul(qT_p[:], lhsT=Wq_sb[0], rhs=qT_in[0], start=True, stop=False)
    nc.tensor.matmul(qT_p[:], lhsT=Wq_sb[1], rhs=qT_in[1], start=False, stop=True)
    # stays f32: read back as the per-partition scalar operand of the adds
    qTf = const.tile([H, ROWS], F32, tag="qTf")
    nc.vector.tensor_copy(qTf[:], qT_p[:])

    # Main loop, q-major: feat[h, (lq, lk)] = tanh(kT[h, :] + qT[h, lq]) with
    # the whole kT row-block as the streamed operand (FD=512 per DVE op) and
    # the q column as per-partition scalar; tanh IN PLACE on big chunks.
    # Then one column-matvec per (lq, lk-block): lhsT = feat slice [h, 128
    # contiguous lk], rhs = wv -> out [128(lk), 1] = column lq of scoresT
    # tile t. Every chunk buffer is never reused (no WAR/WAW waits; walrus
    # only allows one sync-wait per instruction).
    scoresT_ps = []
    for t in range(4):
        sc = acc_ps.tile([128, ROWS], F32, tag=f"scT{t}")
        scoresT_ps.append(sc)
    lq0 = 0
    for c, gsz in enumerate(CHUNKS):
        feat = work.tile([H, gsz * LK], BF16, tag=f"ch{c}")
        for g in range(gsz):
            lq = lq0 + g
            nc.vector.tensor_scalar_add(
                feat[:, LK * g : LK * (g + 1)], kTb[:], qTf[:, lq : lq + 1]
            )
        nc.scalar.activation(feat[:], feat[:], AF.Tanh)
        for t in range(4):
            for g in range(gsz):
                lq = lq0 + g
                nc.tensor.matmul(
                    scoresT_ps[t][0:128, lq : lq + 1],
                    lhsT=feat[:, LK * g + 128 * t : LK * g + 128 * (t + 1)],
                    rhs=wv_bf,
                    start=True,
                    stop=True,
                )
        lq0 += gsz

    # pT = exp(scoresT) directly in transposed layout (tanh-bounded scores
    # can't overflow exp, and softmax's max-shift cancels in p/sum, so no
    # row-max pass is needed).
    pT_sb = []
    for t in range(4):
        s = work.tile([128, ROWS], BF16, tag=f"pT_sb{t}")
        nc.scalar.activation(s[:], scoresT_ps[t][:], AF.Exp)
        pT_sb.append(s)

    out_ps = acc_ps.tile([ROWS, DV], F32, tag="out_ps")
    for t in range(4):
        nc.tensor.matmul(out_ps[:], lhsT=pT_sb[t][:], rhs=vb[t][:], start=(t == 0), stop=(t == 3))
    sum_ps = acc_ps.tile([ROWS, 1], F32, tag="sum_ps")
    for t in range(4):
        nc.tensor.matmul(sum_ps[:], lhsT=pT_sb[t][:], rhs=mb[t][:], start=(t == 0), stop=(t == 3))

    rs = const.tile([ROWS, 1], F32, tag="rs")
    nc.vector.reciprocal(rs[:], sum_ps[:])
    out_sb = const.tile([ROWS, DV], F32, tag="out_sb")
    nc.vector.tensor_scalar_mul(out_sb[:], out_ps[:], rs[:, 0:1])
    nc.sync.dma_start(aps["out"][:, :], out_sb[:])


def build_graph() -> bass.Bass:
    nc = bass.Bass("TRN2", target_bir_lowering=False, debug=False)

    def inp(name, shape):
        return nc.dram_tensor(name, shape, F32, kind="ExternalInput").ap()

    aps = {
        "blob": nc.dram_tensor("blob", [128, BLOB_COLS], BF16, kind="ExternalInput").ap(),
        "out": nc.dram_tensor("out", [ROWS, DV], F32, kind="ExternalOutput").ap(),
    }
    with tile.TileContext(nc) as tc:
        with ExitStack() as ctx:
            _body(ctx, tc, aps)
    _split_multi_waits(nc)
    return nc


def _split_multi_waits(nc):
    """This walrus build accepts only ONE sync-wait per instruction (every
    TPB struct's setupSyncWait rejects more). Tile emits instructions with
    several waits. Legalize: keep one wait on the instruction and hoist the
    rest onto freshly inserted same-engine NOPs placed immediately before it
    in the basic block — identical blocking semantics, no reordering."""
    n = 0
    for bb in nc.m.functions[0].blocks:
        insts = bb.instructions
        out = []
        for inst in insts:
            si = inst.sync_info
            if si is not None and si.on_wait and len(si.on_wait) > 1:
                waits = list(si.on_wait)
                for w in waits[:-1]:
                    nop = mybir.InstNoOp(
                        name=f"{inst.name}-wsplit{n}",
                        text_hint="waitsplit",
                        bass_nofuse=True,
                        engine=inst.engine,
                        sync_info=mybir.SyncInfo(on_wait=[w], on_update=[]),
                    )
                    nc.register_instruction(nop)
                    out.append(nop)
                    n += 1
                inst.sync_info = mybir.SyncInfo(
                    on_wait=[waits[-1]], on_update=si.on_update
                )
            out.append(inst)
        if n:
            bb.instructions = out


def make_in_maps(queries, keys, values, Wq, Wk, wv, valid_lens):
    import ml_dtypes

    bf = ml_dtypes.bfloat16
    f = np.float32
    queries = np.asarray(queries, f)
    keys = np.asarray(keys, f)
    values = np.asarray(values, f)
    Wqf = np.asarray(Wq, f)
    Wkf = np.asarray(Wk, f)
    wvf = np.asarray(wv, f).reshape(H)
    in_maps = []
    for c in range(NCORES):
        b, r0 = c // 2, (c % 2) * ROWS
        m = (np.arange(LK) < int(valid_lens[b])).astype(f)

        a = np.empty((128, BLOB_COLS), f)
        qT = queries[b, r0 : r0 + ROWS, :].T  # [DQ, ROWS]
        kT = keys[b].T  # [DK, LK]
        a[:, 0:128] = qT[0:128]
        a[:, 128:256] = qT[128:256]
        a[:, 256:768] = kT[0:128]
        a[:, 768:1280] = kT[128:256]
        a[:, 1280:1408] = Wqf[0:128]
        a[:, 1408:1536] = Wqf[128:256]
        a[:, 1536:1664] = Wkf[0:128]
        a[:, 1664:1792] = Wkf[128:256]
        a[:, 1792] = wvf
        for t in range(4):
            a[:, 1793 + 256 * t : 1793 + 256 * (t + 1)] = values[b, 128 * t : 128 * (t + 1), :]
            a[:, 2817 + t] = m[128 * t : 128 * (t + 1)]

        in_maps.append({"blob": a.astype(bf)})
    return in_maps


_CACHE: dict = {}


def kernel(queries, keys, values, Wq, Wk, wv, valid_lens, _trace=False, _trace_kwargs=None):
    if "nc" not in _CACHE:
        _CACHE["nc"] = build_graph()
    nc = _CACHE["nc"]
    in_maps = make_in_maps(queries, keys, values, Wq, Wk, wv, valid_lens)
    res = bass_utils.run_bass_kernel_spmd(
        nc,
        in_maps,
        core_ids=list(range(NCORES)),
        trace=_trace,
        **(_trace_kwargs or {}),
    )
    out = np.empty((B, LQ, DV), dtype=np.float32)
    for c in range(NCORES):
        b, r0 = c // 2, (c % 2) * ROWS
        out[b, r0 : r0 + ROWS, :] = res.results[c]["out"]
    if _trace:
        return out, res
    return out


# revision 28
# speedup vs baseline: 2.0271x; 1.3573x over previous
"""AdditiveAttention (d2l-style) distributed Bass kernel for 8 TRN2 NeuronCores.

Full inputs in, full output out.

Sharding (balanced, valid-length aware): batches are sorted by their k-tile
count nk_b = ceil(valid_len/128) descending; every core takes query rows
[32c, 32c+32) of EVERY batch. All 8 cores then carry identical-shape work
(SPMD), and key tiles beyond each batch's valid length are skipped entirely —
the in-tile remainder is handled by a 0/1 mask column. The graph is built per
sorted nk tuple (cached); (4,4,4,4) is the dense case.

Per-core pipeline (all fp32 accumulation, bf16 data path):
  qT[h,lq] = (q @ Wq).T, kT_b[h,lk] = (k_b @ Wk).T     (PE, bf16, d-tiled)
  feat[h, (lq,lk)] = tanh(kT_b[h,:] + qT[h,lq])        (DVE adds, q col as
                                                        per-partition scalar;
                                                        ACT tanh in place on
                                                        big chunks)
  scoresT_t[lk, lq] = wv . feat[:, lq, lk-tile t]      (PE: feat block
                                                        stationary, wv moving,
                                                        N=1 -> one scoresT
                                                        column per matmul)
  pT_t = exp(scoresT_t)     (no max subtraction needed: |scores| <= sum|wv|,
                             exp can't overflow; softmax shift cancels)
  out = (pT.T @ (v ⊙ m)) / (pT.T @ m)                  (per-batch 32-row
                                                        accumulated matmuls at
                                                        col_grp 32*bi)

Masking is exactly equivalent to the reference's -1e6 additive mask: excluded
key positions contribute 0 to both numerator and denominator.
"""

import math
import sys

sys.path.insert(0, "/opt/trn_rl_repo")

from contextlib import ExitStack

import numpy as np

import concourse.bass as bass
import concourse.mybir as mybir
from concourse import bass_utils, tile

B, LQ, LK, DQ, DK, DV, H = 4, 256, 512, 256, 256, 256, 128
NCORES = 8
RPB = 32  # query rows per (core, batch)
F32 = mybir.dt.float32
BF16 = mybir.dt.bfloat16
AF = mybir.ActivationFunctionType


def _blob_layout(nks):
    """Column offsets of the packed all-bf16 input blob for a given sorted
    nk tuple. One blob -> one DMA -> one semaphore (walrus accepts only one
    sync-wait per instruction, so input fan-in must come from one queue)."""
    nktot = sum(nks)
    off = {}
    o = 0
    off["qT"] = o
    o += 2 * 128  # [DQ=2x128 partitions, 128 lq cols]
    off["kT0"] = o
    o += nktot * 128  # d-tile 0, per-batch segments
    off["kT1"] = o
    o += nktot * 128
    off["Wq"] = o
    o += 2 * H
    off["Wk"] = o
    o += 2 * H
    off["wv"] = o
    o += 1
    off["v"] = o
    o += nktot * DV  # per (batch, tile)
    off["m"] = o
    o += nktot  # per (batch, tile)
    return off, o


def _row_chunks(bi, nbatch):
    """Row-chunk plan for batch bi: small leading chunks so the first tanh
    starts early; small trailing chunks so the exp/output tail starts early."""
    if bi == 0:
        return [4, 4, 8, 16]
    if bi == nbatch - 1:
        return [16, 8, 4, 4]
    return [16, 16]


def _body(ctx: ExitStack, tc: "tile.TileContext", aps: dict, nks):
    nc = tc.nc
    nktot = sum(nks)
    ntiles = nks[0]
    segs = [sum(nks[:i]) for i in range(len(nks))]  # k-tile offset per batch
    off, blob_cols = _blob_layout(nks)

    const = ctx.enter_context(tc.tile_pool(name="const", bufs=1))
    work = ctx.enter_context(tc.tile_pool(name="work", bufs=1))
    proj_ps = ctx.enter_context(tc.tile_pool(name="proj_ps", bufs=2, space="PSUM"))
    acc_ps = ctx.enter_context(tc.tile_pool(name="acc_ps", bufs=1, space="PSUM"))

    blob = const.tile([128, blob_cols], BF16, tag="blob")
    nc.sync.dma_start(blob[:], aps["blob"][:, :])

    def slab(o, w):
        return blob[:, o : o + w]

    qT_in = [slab(off["qT"], 128), slab(off["qT"] + 128, 128)]
    kT_in = [slab(off["kT0"], nktot * 128), slab(off["kT1"], nktot * 128)]
    Wq_sb = [slab(off["Wq"], H), slab(off["Wq"] + H, H)]
    Wk_sb = [slab(off["Wk"], H), slab(off["Wk"] + H, H)]
    wv_bf = slab(off["wv"], 1)

    # Projections first (they gate the whole pipeline).
    qT_p = proj_ps.tile([H, 128], F32, tag="projk")
    nc.tensor.matmul(qT_p[:], lhsT=Wq_sb[0], rhs=qT_in[0], start=True, stop=False)
    nc.tensor.matmul(qT_p[:], lhsT=Wq_sb[1], rhs=qT_in[1], start=False, stop=True)
    # stays f32: read back as the per-partition scalar operand of the adds
    qTf = const.tile([H, 128], F32, tag="qTf")
    nc.vector.tensor_copy(qTf[:], qT_p[:])

    kTb = const.tile([H, nktot * 128], BF16, tag="kTb")
    for bi, nk in enumerate(nks):
        w = nk * 128
        kT_p = proj_ps.tile([H, 512], F32, tag="projk")
        nc.tensor.matmul(
            kT_p[:, 0:w], lhsT=Wk_sb[0], rhs=kT_in[0][:, segs[bi] * 128 : segs[bi] * 128 + w],
            start=True, stop=False,
        )
        nc.tensor.matmul(
            kT_p[:, 0:w], lhsT=Wk_sb[1], rhs=kT_in[1][:, segs[bi] * 128 : segs[bi] * 128 + w],
            start=False, stop=True,
        )
        nc.vector.tensor_copy(kTb[:, segs[bi] * 128 : segs[bi] * 128 + w], kT_p[:, 0:w])

    # Main loop: per sorted batch, per row-chunk: DVE adds (FD = nk*128,
    # q col as per-partition scalar), tanh IN PLACE, then one column-matvec
    # per (row, k-tile). Chunk buffers are never reused, so no WAR/WAW waits.
    scoresT_ps = []
    for t in range(ntiles):
        sc = acc_ps.tile([128, 128], F32, tag=f"scT{t}")
        scoresT_ps.append(sc)

    nchunk = 0
    for bi, nk in enumerate(nks):
        fd = nk * 128
        kslab = kTb[:, segs[bi] * 128 : segs[bi] * 128 + fd]
        r0 = 0
        for gsz in _row_chunks(bi, len(nks)):
            feat = work.tile([H, gsz * fd], BF16, tag=f"ch{nchunk}")
            nchunk += 1
            for g in range(gsz):
                lq = RPB * bi + r0 + g
                nc.vector.tensor_scalar_add(
                    feat[:, fd * g : fd * (g + 1)], kslab, qTf[:, lq : lq + 1]
                )
            nc.scalar.activation(feat[:], feat[:], AF.Tanh)
            for t in range(nk):
                for g in range(gsz):
                    lq = RPB * bi + r0 + g
                    nc.tensor.matmul(
                        scoresT_ps[t][0:128, lq : lq + 1],
                        lhsT=feat[:, fd * g + 128 * t : fd * g + 128 * (t + 1)],
                        rhs=wv_bf,
                        start=True,
                        stop=True,
                    )
            r0 += gsz

    # masked values (low priority: scheduled into DVE idle slots)
    vb, mb = [], []
    for i in range(nktot):
        mcol = slab(off["m"] + i, 1)
        x = const.tile([128, DV], BF16, tag=f"vb{i}")
        nc.vector.tensor_tensor(
            out=x[:],
            in0=slab(off["v"] + i * DV, DV),
            in1=mcol.broadcast_to([128, DV]),
            op=mybir.AluOpType.mult,
        )
        vb.append(x)
        mb.append(mcol)

    # pT_t = exp(scoresT_t) on the valid column prefix (batches are sorted by
    # nk desc, so tiles beyond a batch's nk form an untouched suffix).
    pT_sb = []
    for t in range(ntiles):
        valid = RPB * sum(1 for x in nks if x > t)
        s = work.tile([128, 128], BF16, tag=f"pT{t}")
        nc.scalar.activation(s[:, 0:valid], scoresT_ps[t][:, 0:valid], AF.Exp)
        pT_sb.append(s)

    # out[32bi:32bi+32, :] = sum_t pT_t[:, block].T @ vb ; denominator via m
    out_ps = acc_ps.tile([128, DV], F32, tag="out_ps")
    sum_ps = acc_ps.tile([128, 1], F32, tag="sum_ps")
    for bi, nk in enumerate(nks):
        sl = slice(RPB * bi, RPB * bi + RPB)
        for t in range(nk):
            nc.tensor.matmul(
                out_ps[sl, :],
                lhsT=pT_sb[t][:, sl],
                rhs=vb[segs[bi] + t][:],
                start=(t == 0),
                stop=(t == nk - 1),
                tile_position=(0, RPB * bi),
            )
        for t in range(nk):
            nc.tensor.matmul(
                sum_ps[sl, 0:1],
                lhsT=pT_sb[t][:, sl],
                rhs=mb[segs[bi] + t],
                start=(t == 0),
                stop=(t == nk - 1),
                tile_position=(0, RPB * bi),
            )

    rs = const.tile([128, 1], F32, tag="rs")
    nc.vector.reciprocal(rs[:], sum_ps[:])
    out_sb = const.tile([128, DV], F32, tag="out_sb")
    nc.vector.tensor_scalar_mul(out_sb[:], out_ps[:], rs[:, 0:1])
    nc.sync.dma_start(aps["out"][:, :], out_sb[:])


def build_graph(nks) -> bass.Bass:
    nc = bass.Bass("TRN2", target_bir_lowering=False, debug=False)
    _, blob_cols = _blob_layout(nks)
    aps = {
        "blob": nc.dram_tensor("blob", [128, blob_cols], BF16, kind="ExternalInput").ap(),
        "out": nc.dram_tensor("out", [128, DV], F32, kind="ExternalOutput").ap(),
    }
    with tile.TileContext(nc) as tc:
        with ExitStack() as ctx:
            _body(ctx, tc, aps, nks)
    _split_multi_waits(nc)
    return nc


def _split_multi_waits(nc):
    """This walrus build accepts only ONE sync-wait per instruction (every
    TPB struct's setupSyncWait rejects more). Tile emits instructions with
    several waits. Legalize: keep one wait on the instruction and hoist the
    rest onto freshly inserted same-engine NOPs placed immediately before it
    in the basic block — identical blocking semantics, no reordering."""
    n = 0
    for bb in nc.m.functions[0].blocks:
        insts = bb.instructions
        out = []
        for inst in insts:
            si = inst.sync_info
            if si is not None and si.on_wait and len(si.on_wait) > 1:
                waits = list(si.on_wait)
                for w in waits[:-1]:
                    nop = mybir.InstNoOp(
                        name=f"{inst.name}-wsplit{n}",
                        text_hint="waitsplit",
                        bass_nofuse=True,
                        engine=inst.engine,
                        sync_info=mybir.SyncInfo(on_wait=[w], on_update=[]),
                    )
                    nc.register_instruction(nop)
                    out.append(nop)
                    n += 1
                inst.sync_info = mybir.SyncInfo(
                    on_wait=[waits[-1]], on_update=si.on_update
                )
            out.append(inst)
        if n:
            bb.instructions = out


def _plan(valid_lens):
    nk = [min(4, max(1, math.ceil(int(v) / 128))) for v in valid_lens]
    order = sorted(range(B), key=lambda b: -nk[b])
    nks = tuple(nk[b] for b in order)
    return order, nks


def make_in_maps(queries, keys, values, Wq, Wk, wv, valid_lens, order, nks):
    import ml_dtypes

    bf = ml_dtypes.bfloat16
    f = np.float32
    queries = np.asarray(queries, f)
    keys = np.asarray(keys, f)
    values = np.asarray(values, f)
    Wqf = np.asarray(Wq, f)
    Wkf = np.asarray(Wk, f)
    wvf = np.asarray(wv, f).reshape(H)
    off, blob_cols = _blob_layout(nks)
    segs = [sum(nks[:i]) for i in range(len(nks))]

    base = np.empty((128, blob_cols), f)  # core-independent part
    base[:, off["Wq"] : off["Wq"] + H] = Wqf[0:128]
    base[:, off["Wq"] + H : off["Wq"] + 2 * H] = Wqf[128:256]
    base[:, off["Wk"] : off["Wk"] + H] = Wkf[0:128]
    base[:, off["Wk"] + H : off["Wk"] + 2 * H] = Wkf[128:256]
    base[:, off["wv"]] = wvf
    for bi, b in enumerate(order):
        nk = nks[bi]
        kT = keys[b].T  # [DK, LK]
        m = (np.arange(LK) < int(valid_lens[b])).astype(f)
        s = segs[bi] * 128
        base[:, off["kT0"] + s : off["kT0"] + s + nk * 128] = kT[0:128, : nk * 128]
        base[:, off["kT1"] + s : off["kT1"] + s + nk * 128] = kT[128:256, : nk * 128]
        for t in range(nk):
            base[:, off["v"] + (segs[bi] + t) * DV : off["v"] + (segs[bi] + t + 1) * DV] = values[
                b, 128 * t : 128 * (t + 1), :
            ]
            base[:, off["m"] + segs[bi] + t] = m[128 * t : 128 * (t + 1)]

    in_maps = []
    for c in range(NCORES):
        a = base.copy()
        for bi, b in enumerate(order):
            qT = queries[b, RPB * c : RPB * (c + 1), :].T  # [DQ, 32]
            a[:, off["qT"] + RPB * bi : off["qT"] + RPB * (bi + 1)] = qT[0:128]
            a[:, off["qT"] + 128 + RPB * bi : off["qT"] + 128 + RPB * (bi + 1)] = qT[128:256]
        in_maps.append({"blob": a.astype(bf)})
    return in_maps


_CACHE: dict = {}


def kernel(queries, keys, values, Wq, Wk, wv, valid_lens, _trace=False, _trace_kwargs=None):
    order, nks = _plan(valid_lens)
    if nks not in _CACHE:
        _CACHE[nks] = build_graph(nks)
    nc = _CACHE[nks]
    in_maps = make_in_maps(queries, keys, values, Wq, Wk, wv, valid_lens, order, nks)
    res = bass_utils.run_bass_kernel_spmd(
        nc,
        in_maps,
        core_ids=list(range(NCORES)),
        trace=_trace,
        **(_trace_kwargs or {}),
    )
    out = np.empty((B, LQ, DV), dtype=np.float32)
    for c in range(NCORES):
        o = res.results[c]["out"]
        for bi, b in enumerate(order):
            out[b, RPB * c : RPB * (c + 1), :] = o[RPB * bi : RPB * (bi + 1), :]
    if _trace:
        return out, res
    return out


# revision 34
# speedup vs baseline: 2.0708x; 1.0215x over previous
"""AdditiveAttention (d2l-style) distributed Bass kernel for 8 TRN2 NeuronCores.

Full inputs in, full output out.

Sharding (balanced, valid-length aware): batches are sorted by their k-tile
count nk_b = ceil(valid_len/128) descending; every core takes query rows
[32c, 32c+32) of EVERY batch. All 8 cores then carry identical-shape work
(SPMD), and key tiles beyond each batch's valid length are skipped entirely —
the in-tile remainder is handled by a 0/1 mask column. The graph is built per
sorted nk tuple (cached); (4,4,4,4) is the dense case.

Per-core pipeline (all fp32 accumulation, bf16 data path):
  qT[h,lq] = (q @ Wq).T, kT_b[h,lk] = (k_b @ Wk).T     (PE, bf16, d-tiled)
  feat[h, (lq,lk)] = tanh(kT_b[h,:] + qT[h,lq])        (DVE adds, q col as
                                                        per-partition scalar;
                                                        ACT tanh in place on
                                                        big chunks)
  scoresT_t[lk, lq] = wv . feat[:, lq, lk-tile t]      (PE: feat block
                                                        stationary, wv moving,
                                                        N=1 -> one scoresT
                                                        column per matmul)
  pT_t = exp(scoresT_t)     (no max subtraction needed: |scores| <= sum|wv|,
                             exp can't overflow; softmax shift cancels)
  out = (pT.T @ (v ⊙ m)) / (pT.T @ m)                  (per-batch 32-row
                                                        accumulated matmuls at
                                                        col_grp 32*bi)

Masking is exactly equivalent to the reference's -1e6 additive mask: excluded
key positions contribute 0 to both numerator and denominator.
"""

import math
import sys

sys.path.insert(0, "/opt/trn_rl_repo")

from contextlib import ExitStack

import numpy as np

import concourse.bass as bass
import concourse.mybir as mybir
from concourse import bass_utils, tile

B, LQ, LK, DQ, DK, DV, H = 4, 256, 512, 256, 256, 256, 128
NCORES = 8
RPB = 32  # query rows per (core, batch)
F32 = mybir.dt.float32
BF16 = mybir.dt.bfloat16
AF = mybir.ActivationFunctionType


def _blob_layout(nks):
    """Column offsets of the two packed all-bf16 input blobs for a given
    sorted nk tuple. Each blob is one DMA on its own queue/semaphore with a
    disjoint consumer set (walrus accepts only one sync-wait per instruction,
    so no instruction may depend on both queues). The main blob gates the
    compute ramp; the vals blob is only needed for the output tail."""
    nktot = sum(nks)
    off = {}
    o = 0
    off["qT"] = o
    o += 2 * 128  # [DQ=2x128 partitions, 128 lq cols]
    off["kT0"] = o
    o += nktot * 128  # d-tile 0, per-batch segments
    off["kT1"] = o
    o += nktot * 128
    off["Wq"] = o
    o += 2 * H
    off["Wk"] = o
    o += 2 * H
    off["wv"] = o
    o += 1
    main_cols = o
    o = 0
    off["v"] = o
    o += nktot * DV  # per (batch, tile)
    off["m"] = o
    o += nktot  # per (batch, tile)
    return off, main_cols, o


def _row_chunks(bi, nbatch):
    """Row-chunk plan for batch bi: small leading chunks so the first tanh
    starts early; small trailing chunks so the exp/output tail starts early."""
    if bi == 0:
        return [4, 4, 8, 16]
    if bi == nbatch - 1:
        return [16, 8, 4, 4]
    return [16, 16]


def _body(ctx: ExitStack, tc: "tile.TileContext", aps: dict, nks):
    nc = tc.nc
    nktot = sum(nks)
    ntiles = nks[0]
    segs = [sum(nks[:i]) for i in range(len(nks))]  # k-tile offset per batch
    off, main_cols, vals_cols = _blob_layout(nks)

    const = ctx.enter_context(tc.tile_pool(name="const", bufs=1))
    work = ctx.enter_context(tc.tile_pool(name="work", bufs=1))
    proj_ps = ctx.enter_context(tc.tile_pool(name="proj_ps", bufs=2, space="PSUM"))
    acc_ps = ctx.enter_context(tc.tile_pool(name="acc_ps", bufs=1, space="PSUM"))

    blob = const.tile([128, main_cols], BF16, tag="blob")
    nc.sync.dma_start(blob[:], aps["blob"][:, :])
    vblob = const.tile([128, vals_cols], BF16, tag="vblob")
    nc.sync.dma_start(vblob[:], aps["vblob"][:, :])

    def slab(o, w):
        return blob[:, o : o + w]

    def vslab(o, w):
        return vblob[:, o : o + w]

    qT_in = [slab(off["qT"], 128), slab(off["qT"] + 128, 128)]
    kT_in = [slab(off["kT0"], nktot * 128), slab(off["kT1"], nktot * 128)]
    Wq_sb = [slab(off["Wq"], H), slab(off["Wq"] + H, H)]
    Wk_sb = [slab(off["Wk"], H), slab(off["Wk"] + H, H)]
    wv_bf = slab(off["wv"], 1)

    # Projections first (they gate the whole pipeline).
    qT_p = proj_ps.tile([H, 128], F32, tag="projk")
    nc.tensor.matmul(qT_p[:], lhsT=Wq_sb[0], rhs=qT_in[0], start=True, stop=False)
    nc.tensor.matmul(qT_p[:], lhsT=Wq_sb[1], rhs=qT_in[1], start=False, stop=True)
    # stays f32: read back as the per-partition scalar operand of the adds
    qTf = const.tile([H, 128], F32, tag="qTf")
    nc.vector.tensor_copy(qTf[:], qT_p[:])

    kTb = const.tile([H, nktot * 128], BF16, tag="kTb")
    for bi, nk in enumerate(nks):
        w = nk * 128
        kT_p = proj_ps.tile([H, 512], F32, tag="projk")
        nc.tensor.matmul(
            kT_p[:, 0:w], lhsT=Wk_sb[0], rhs=kT_in[0][:, segs[bi] * 128 : segs[bi] * 128 + w],
            start=True, stop=False,
        )
        nc.tensor.matmul(
            kT_p[:, 0:w], lhsT=Wk_sb[1], rhs=kT_in[1][:, segs[bi] * 128 : segs[bi] * 128 + w],
            start=False, stop=True,
        )
        nc.vector.tensor_copy(kTb[:, segs[bi] * 128 : segs[bi] * 128 + w], kT_p[:, 0:w])

    # Main loop: per sorted batch, per row-chunk: DVE adds (FD = nk*128,
    # q col as per-partition scalar), tanh IN PLACE, then one column-matvec
    # per (row, k-tile). Chunk buffers are never reused, so no WAR/WAW waits.
    scoresT_ps = []
    for t in range(ntiles):
        sc = acc_ps.tile([128, 128], F32, tag=f"scT{t}")
        scoresT_ps.append(sc)

    nchunk = 0
    for bi, nk in enumerate(nks):
        fd = nk * 128
        kslab = kTb[:, segs[bi] * 128 : segs[bi] * 128 + fd]
        r0 = 0
        for gsz in _row_chunks(bi, len(nks)):
            feat = work.tile([H, gsz * fd], BF16, tag=f"ch{nchunk}")
            nchunk += 1
            for g in range(gsz):
                lq = RPB * bi + r0 + g
                nc.vector.tensor_scalar_add(
                    feat[:, fd * g : fd * (g + 1)], kslab, qTf[:, lq : lq + 1]
                )
            nc.scalar.activation(feat[:], feat[:], AF.Tanh)
            for t in range(nk):
                for g in range(gsz):
                    lq = RPB * bi + r0 + g
                    nc.tensor.matmul(
                        scoresT_ps[t][0:128, lq : lq + 1],
                        lhsT=feat[:, fd * g + 128 * t : fd * g + 128 * (t + 1)],
                        rhs=wv_bf,
                        start=True,
                        stop=True,
                    )
            r0 += gsz

    # masked values (low priority: scheduled into DVE idle slots)
    vb, mb = [], []
    for i in range(nktot):
        mcol = vslab(off["m"] + i, 1)
        x = const.tile([128, DV], BF16, tag=f"vb{i}")
        nc.vector.tensor_tensor(
            out=x[:],
            in0=vslab(off["v"] + i * DV, DV),
            in1=mcol.broadcast_to([128, DV]),
            op=mybir.AluOpType.mult,
        )
        vb.append(x)
        mb.append(mcol)

    # pT_t = exp(scoresT_t) on the valid column prefix (batches are sorted by
    # nk desc, so tiles beyond a batch's nk form an untouched suffix).
    pT_sb = []
    for t in range(ntiles):
        valid = RPB * sum(1 for x in nks if x > t)
        s = work.tile([128, 128], BF16, tag=f"pT{t}")
        nc.scalar.activation(s[:, 0:valid], scoresT_ps[t][:, 0:valid], AF.Exp)
        pT_sb.append(s)

    # out[32bi:32bi+32, :] = sum_t pT_t[:, block].T @ vb ; denominator via m
    out_ps = acc_ps.tile([128, DV], F32, tag="out_ps")
    sum_ps = acc_ps.tile([128, 1], F32, tag="sum_ps")
    for bi, nk in enumerate(nks):
        sl = slice(RPB * bi, RPB * bi + RPB)
        for t in range(nk):
            nc.tensor.matmul(
                out_ps[sl, :],
                lhsT=pT_sb[t][:, sl],
                rhs=vb[segs[bi] + t][:],
                start=(t == 0),
                stop=(t == nk - 1),
                tile_position=(0, RPB * bi),
            )
        for t in range(nk):
            nc.tensor.matmul(
                sum_ps[sl, 0:1],
                lhsT=pT_sb[t][:, sl],
                rhs=mb[segs[bi] + t],
                start=(t == 0),
                stop=(t == nk - 1),
                tile_position=(0, RPB * bi),
            )

    rs = const.tile([128, 1], F32, tag="rs")
    nc.vector.reciprocal(rs[:], sum_ps[:])
    out_sb = const.tile([128, DV], F32, tag="out_sb")
    nc.vector.tensor_scalar_mul(out_sb[:], out_ps[:], rs[:, 0:1])
    nc.sync.dma_start(aps["out"][:, :], out_sb[:])


def build_graph(nks) -> bass.Bass:
    nc = bass.Bass("TRN2", target_bir_lowering=False, debug=False)
    _, main_cols, vals_cols = _blob_layout(nks)
    aps = {
        "blob": nc.dram_tensor("blob", [128, main_cols], BF16, kind="ExternalInput").ap(),
        "vblob": nc.dram_tensor("vblob", [128, vals_cols], BF16, kind="ExternalInput").ap(),
        "out": nc.dram_tensor("out", [128, DV], F32, kind="ExternalOutput").ap(),
    }
    with tile.TileContext(nc) as tc:
        with ExitStack() as ctx:
            _body(ctx, tc, aps, nks)
    _split_multi_waits(nc)
    return nc


def _split_multi_waits(nc):
    """This walrus build accepts only ONE sync-wait per instruction (every
    TPB struct's setupSyncWait rejects more). Tile emits instructions with
    several waits. Legalize: keep one wait on the instruction and hoist the
    rest onto freshly inserted same-engine NOPs placed immediately before it
    in the basic block — identical blocking semantics, no reordering."""
    n = 0
    for bb in nc.m.functions[0].blocks:
        insts = bb.instructions
        out = []
        for inst in insts:
            si = inst.sync_info
            if si is not None and si.on_wait and len(si.on_wait) > 1:
                waits = list(si.on_wait)
                for w in waits[:-1]:
                    nop = mybir.InstNoOp(
                        name=f"{inst.name}-wsplit{n}",
                        text_hint="waitsplit",
                        bass_nofuse=True,
                        engine=inst.engine,
                        sync_info=mybir.SyncInfo(on_wait=[w], on_update=[]),
                    )
                    nc.register_instruction(nop)
                    out.append(nop)
                    n += 1
                inst.sync_info = mybir.SyncInfo(
                    on_wait=[waits[-1]], on_update=si.on_update
                )
            out.append(inst)
        if n:
            bb.instructions = out


def _plan(valid_lens):
    nk = [min(4, max(1, math.ceil(int(v) / 128))) for v in valid_lens]
    order = sorted(range(B), key=lambda b: -nk[b])
    nks = tuple(nk[b] for b in order)
    return order, nks


def make_in_maps(queries, keys, values, Wq, Wk, wv, valid_lens, order, nks):
    import ml_dtypes

    bf = ml_dtypes.bfloat16
    f = np.float32
    queries = np.asarray(queries, f)
    keys = np.asarray(keys, f)
    values = np.asarray(values, f)
    Wqf = np.asarray(Wq, f)
    Wkf = np.asarray(Wk, f)
    wvf = np.asarray(wv, f).reshape(H)
    off, main_cols, vals_cols = _blob_layout(nks)
    segs = [sum(nks[:i]) for i in range(len(nks))]

    base = np.empty((128, main_cols), f)  # core-independent part
    base[:, off["Wq"] : off["Wq"] + H] = Wqf[0:128]
    base[:, off["Wq"] + H : off["Wq"] + 2 * H] = Wqf[128:256]
    base[:, off["Wk"] : off["Wk"] + H] = Wkf[0:128]
    base[:, off["Wk"] + H : off["Wk"] + 2 * H] = Wkf[128:256]
    base[:, off["wv"]] = wvf
    vbase = np.empty((128, vals_cols), f)
    for bi, b in enumerate(order):
        nk = nks[bi]
        kT = keys[b].T  # [DK, LK]
        m = (np.arange(LK) < int(valid_lens[b])).astype(f)
        s = segs[bi] * 128
        base[:, off["kT0"] + s : off["kT0"] + s + nk * 128] = kT[0:128, : nk * 128]
        base[:, off["kT1"] + s : off["kT1"] + s + nk * 128] = kT[128:256, : nk * 128]
        for t in range(nk):
            i = segs[bi] + t
            vbase[:, off["v"] + i * DV : off["v"] + (i + 1) * DV] = values[
                b, 128 * t : 128 * (t + 1), :
            ]
            vbase[:, off["m"] + i] = m[128 * t : 128 * (t + 1)]

    in_maps = []
    for c in range(NCORES):
        a = base.copy()
        for bi, b in enumerate(order):
            qT = queries[b, RPB * c : RPB * (c + 1), :].T  # [DQ, 32]
            a[:, off["qT"] + RPB * bi : off["qT"] + RPB * (bi + 1)] = qT[0:128]
            a[:, off["qT"] + 128 + RPB * bi : off["qT"] + 128 + RPB * (bi + 1)] = qT[128:256]
        in_maps.append({"blob": a.astype(bf), "vblob": vbase.astype(bf)})
    return in_maps


_CACHE: dict = {}


def kernel(queries, keys, values, Wq, Wk, wv, valid_lens, _trace=False, _trace_kwargs=None):
    order, nks = _plan(valid_lens)
    if nks not in _CACHE:
        _CACHE[nks] = build_graph(nks)
    nc = _CACHE[nks]
    in_maps = make_in_maps(queries, keys, values, Wq, Wk, wv, valid_lens, order, nks)
    res = bass_utils.run_bass_kernel_spmd(
        nc,
        in_maps,
        core_ids=list(range(NCORES)),
        trace=_trace,
        **(_trace_kwargs or {}),
    )
    out = np.empty((B, LQ, DV), dtype=np.float32)
    for c in range(NCORES):
        o = res.results[c]["out"]
        for bi, b in enumerate(order):
            out[b, RPB * c : RPB * (c + 1), :] = o[RPB * bi : RPB * (bi + 1), :]
    if _trace:
        return out, res
    return out
